# revision 21
# baseline (speedup 1.0000x reference)
"""Trainium2 Bass kernel for nn_FIS_ImportanceAssessment.

Reference computation, per pixel (B=16, C=256, H=W=64):
    sumsq = sum_c f^2 ; sum = sum_c f
    mag   = clip(sqrt(sumsq/C), 0, 1)
    var   = clip((sumsq - sum^2/C)/(C-1), 0, 1)
    grad  = sqrt(var_clipped)               (== clip(sqrt(var), 0, 1))
    out   = sigmoid(relu([mag,var,grad] @ W1 + b1) @ W2 + b2)

Sharding: data-parallel over batch, 2 batches per core across 8 cores.

Per-core layout: the C-axis reduction runs on the PE with "block one-hot"
stationary operands.  The core's 8192 pixels form 16 chunks of 512; chunk
g's column sums land on PSUM partitions {3g, 3g+1, 3g+2} (a "stacked
stats" layout): partition 3g gets only sumsq (mag precursor), 3g+1 and
3g+2 get both sum and sumsq (var / grad precursors).  The tail is then:

    a = sum^2                 (ACT Square; 0 on the 3g rows)
    u = sumsq - a/C           (DVE STT; == sumsq on the 3g rows)
    v = min(alpha*u, 1)       (DVE TS, alpha = 1/C @3g else 1/(C-1))
        -> v[3g]  = clip(sumsq/C)  = mag^2_c,  v[3g+1,2] = var_c
    r = sqrt(v)               (ACT) -> r[3g] = mag_c, r[3g+2] = grad_c
    z_k = W1-block-diag matmul over the stacked (v; r) tile  (PE!)
    h_k = relu(z_k + b1)      (both halves on DVE tensor_scalar)
    out = sigmoid(blockdiag-W2 @ h)                          (PE + ACT)

so the whole 3->16->1 MLP costs 4 matmuls + 2 relu ops instead of a long
per-partition-scalar DVE chain.

ACT table strategy: both relu halves run on DVE, so after the tail sqrt
the ACT engine is idle and the walrus-inserted sigmoid table load (the
only mid-kernel table switch) executes there, off the critical path,
instead of gating the final sigmoid.

Measured (this session): baseline ~47.9 us -> this kernel ~44.1 us.
~10.3 us of that is fixed framework overhead (counted preamble slice +
exit barrier + ~300 event-semaphore resets in the epilogue — present
even for a 3-instruction kernel, measured 15.2 us floor); the feature
stream runs at ~370-410 GB/s, at the per-core HBM roofline.
"""

from contextlib import ExitStack

import numpy as np

import concourse.bacc as bacc
import concourse.bass as bass
import concourse.tile as tile
from concourse import mybir

F32 = mybir.dt.float32
F32R = mybir.dt.float32r  # TF32-style single-pass PE dtype (fp32 is 4 cyc/row)
BF16 = mybir.dt.bfloat16
AF = mybir.ActivationFunctionType
OP = mybir.AluOpType

# -------- problem geometry (hardcoded per contract) --------
B, C, H, W = 16, 256, 64, 64
NCORES = 8
B_PER_CORE = B // NCORES          # 2
PIX = B_PER_CORE * H * W          # 8192 pixels per core
NG = 16                           # pixel chunks ("groups") per core
CHUNK = PIX // NG                 # 512 pixels per chunk (= 1 PSUM bank)
NHID = 16                         # MLP hidden width

# consts_h (bf16) column layout
SQW = 0            # [0:256)    sq one-hot windows (cols 128..130 ones)
Z0COL = 256        # [256:384)  stacked W1 pattern, hidden half 0 (96 rows)
Z1COL = 384        # [384:512)  stacked W1 pattern, hidden half 1
BD0COL = 512       # [512:528)  block-diag W2, half 0
BD1COL = 528       # [528:544)  block-diag W2, half 1
NCONST_H = 544
# consts_f (fp32) column layout
#   0: alpha (1/C on 3g rows, 1/(C-1) on 3g+1,2) ; 1: b1 half0 ; 2: b1 half1
#   3: b2
NCONST_F = 8


PIECES5 = [(0, 1024), (1024, 1024), (2048, 1024), (3072, 512), (3584, 512)]
PIECES6 = [
    (0, 1024),
    (1024, 1024),
    (2048, 1024),
    (3072, 512),
    (3584, 256),
    (3840, 256),
]


def build_nc(pieces=PIECES5, dummy_sig=False, tail_split=False, hk1_dve=False, dual_ring=False, gp_square=False, dummy_sqrt=False, hsplit_first=False) -> bass.Bass:
    # Bacc (not raw Bass): its finalize() runs generate_event_semaphores,
    # which splits multi-sem waits to satisfy the 1-wait-per-instruction
    # hardware constraint that walrus codegen enforces.
    nc = bacc.Bacc()
    # float32r end-to-end for everything the PE consumes: the BIR verifier
    # requires fp32r-matmul inputs to be *produced* as float32r.
    feat = nc.dram_tensor(
        "features", [B_PER_CORE, C, H * W], F32R, kind="ExternalInput"
    )
    cst_r = nc.dram_tensor("consts_r", [128, 256], F32R, kind="ExternalInput")
    cst_h = nc.dram_tensor("consts_h", [128, NCONST_H], BF16, kind="ExternalInput")
    cst_f = nc.dram_tensor("consts_f", [128, NCONST_F], F32, kind="ExternalInput")
    out_d = nc.dram_tensor("out", [NG, CHUNK], F32, kind="ExternalOutput")

    with tile.TileContext(nc) as tc, ExitStack() as ctx:
        singles = ctx.enter_context(tc.tile_pool(name="singles", bufs=1))
        # bufs=2: both streaming rounds get fresh slots, so no x/sq DMA
        # ever carries a buffer-reuse (WAR) wait on top of its RAW wait.
        xpool = ctx.enter_context(tc.tile_pool(name="xpool", bufs=2))
        sqpool = ctx.enter_context(tc.tile_pool(name="sqpool", bufs=2))
        tailp = ctx.enter_context(tc.tile_pool(name="tailp", bufs=1))
        psump = ctx.enter_context(tc.tile_pool(name="psump", bufs=1, space="PSUM"))

        psum_sum = psump.tile([128, CHUNK], F32)
        psum_sq = psump.tile([128, CHUNK], F32)
        psum_z0 = psump.tile([128, CHUNK], F32)
        psum_z1 = psump.tile([128, CHUNK], F32)
        psum2 = psump.tile([NG, CHUNK], F32)

        xs, sqs = [], []
        for b in range(B_PER_CORE):
            xs.append(xpool.tile([128, 2, H * W], F32R, tag="x", name=f"x_{b}"))
            sqs.append(sqpool.tile([128, 2, H * W], BF16, tag="sq", name=f"sq_{b}"))

        # Consts go via SWDGE (gpsimd) so the HWDGE ring is free for the
        # feature stream — features issue immediately after the preamble
        # while the tiny consts land in parallel.
        cons_r = singles.tile([128, 256], F32R)
        nc.gpsimd.dma_start(out=cons_r, in_=cst_r[:])
        cons_h = singles.tile([128, NCONST_H], BF16)
        nc.gpsimd.dma_start(out=cons_h, in_=cst_h[:])
        cons_f = singles.tile([128, NCONST_F], F32)
        nc.gpsimd.dma_start(out=cons_f, in_=cst_f[:])

        if dummy_sqrt:
            # First ACT op = Sqrt: walrus loads the sqrt table set (which
            # also holds square) during the uncounted preamble, so neither
            # the streaming squares nor the tail sqrt pay a table switch.
            # Writes into psum2 (cleared by the start=True output matmul)
            # so DCE can't drop it.
            nc.scalar.activation(psum2[0:2, 2:4], cons_f[0:2, 0:2], AF.Sqrt)

        # Absorb the consts-DMA waits on the PE here so the first real
        # matmuls only wait on the features/squares. (psum2 is cleared again
        # by the real start=True matmul of the MLP output group later.)
        # (2x2, not 1x1: fp32r matmuls require even free dims.)
        nc.tensor.matmul(
            psum2[0:2, 0:2],
            lhsT=cons_r[:, 0:2],
            rhs=cons_r[:, 0:2],
            start=True,
            stop=True,
        )
        nc.tensor.matmul(
            psum2[0:2, 0:2],
            lhsT=cons_h[:, 0:2],
            rhs=cons_h[:, 0:2],
            start=True,
            stop=True,
        )

        # ---- streaming phase: load, square, PE column-sum reductions ----
        # 1 MiB DMA pieces ([128, 2 C-halves, 1024 px]) so compute starts
        # ~3 us in and stays pipelined with the DMA stream.  Squares cast to
        # bf16 (full PE clock + fast weight load on the squared path) and
        # alternate ACT/DVE so no single engine gates the matmul stream.
        sq_engines = {
            (p, h): ("A" if (p + h) % 2 == 0 else "V")
            for p in range(len(pieces))
            for h in range(2)
        }
        # Per-piece matmul sub-plans: (chunk q, column range within the
        # chunk).  256-px pieces produce half-chunk matmuls so the final
        # reductions after the last DMA sem are short.
        def piece_chunks(p0, plen):
            out = []
            for q in range(H * W // CHUNK):
                a0 = max(CHUNK * q, p0)
                b0 = min(CHUNK * (q + 1), p0 + plen)
                if a0 < b0:
                    out.append((q, a0 - CHUNK * q, b0 - CHUNK * q))
            return out

        # batches x C-halves x sub-chunks; same count for the sum and sq paths
        n_mm_per_path = B_PER_CORE * 2 * sum(
            len(piece_chunks(p0, pl)) for p0, pl in pieces
        )
        nsum = 0
        nsq = 0
        npiece = 0
        for b in range(B_PER_CORE):
            x, sq = xs[b], sqs[b]
            feat_b = feat[b].rearrange("(h c) p -> c h p", h=2)
            for p, (p0, plen) in enumerate(pieces):
                psl = slice(p0, p0 + plen)
                # dual_ring: alternate the two physical HWDGE rings (SP and
                # ACT sequencers) so more packets are in flight per SDMA
                # engine.
                deng = nc.scalar if (dual_ring and npiece % 2 == 1) else nc.sync
                if hsplit_first and npiece == 0:
                    # Two per-half DMAs for the very first piece: each has
                    # one 4 KiB run per partition instead of two, so HWDGE
                    # issues it in ~half the time and the ring starts
                    # draining earlier — the whole FIFO stream shifts left.
                    deng.dma_start(out=x[:, 0, psl], in_=feat_b[:, 0, psl])
                    deng.dma_start(out=x[:, 1, psl], in_=feat_b[:, 1, psl])
                else:
                    deng.dma_start(out=x[:, :, psl], in_=feat_b[:, :, psl])
                npiece += 1
                last_piece_sq = (
                    gp_square and b == B_PER_CORE - 1 and p == len(pieces) - 1
                )
                for half in range(2):
                    xin = x[:, half, psl].bitcast(F32)
                    sqo = sq[:, half, psl]
                    if last_piece_sq and half == 1:
                        # 3-way split: DVE + GPSIMD halves run in parallel
                        # with ACT's h0 square, shortening the post-stream
                        # reduction lag.
                        hl = plen // 2
                        nc.vector.tensor_mul(
                            sq[:, 1, p0 : p0 + hl],
                            x[:, 1, p0 : p0 + hl].bitcast(F32),
                            x[:, 1, p0 : p0 + hl].bitcast(F32),
                        )
                        nc.gpsimd.tensor_mul(
                            sq[:, 1, p0 + hl : p0 + plen],
                            x[:, 1, p0 + hl : p0 + plen].bitcast(F32),
                            x[:, 1, p0 + hl : p0 + plen].bitcast(F32),
                        )
                    elif sq_engines[(p, half)] == "A":
                        nc.scalar.activation(sqo, xin, AF.Square)
                    else:
                        nc.vector.tensor_mul(sqo, xin, xin)
                # For the final pieces, issue sum-MMs for BOTH halves before
                # the sq-MMs: sum-MMs only need the DMA, so psum_sum (and the
                # tail's a = sum^2) completes earlier while the final squares
                # still run.  Elsewhere keep sum/sq interleaved (global
                # sum-first ordering stalls the PE FIFO).
                last_piece = b == B_PER_CORE - 1 and p >= len(pieces) - 2
                mm_plan = []
                for half in range(2):
                    for qab in piece_chunks(p0, plen):
                        if last_piece:
                            mm_plan.append(("sum", half, qab))
                        else:
                            mm_plan.append(("sum", half, qab))
                            mm_plan.append(("sq", half, qab))
                if last_piece:
                    for half in range(2):
                        for qab in piece_chunks(p0, plen):
                            mm_plan.append(("sq", half, qab))
                for kind, half, (q, ca, cb) in mm_plan:
                    g = b * (H * W // CHUNK) + q
                    sl = slice(q * CHUNK + ca, q * CHUNK + cb)
                    if kind == "sum":
                        nc.tensor.matmul(
                            psum_sum[:, ca:cb],
                            lhsT=cons_r[:, 128 - 3 * g : 256 - 3 * g],
                            rhs=x[:, half, sl],
                            start=(nsum == 0),
                            stop=(nsum == n_mm_per_path - 1),
                        )
                        nsum += 1
                    else:
                        nc.tensor.matmul(
                            psum_sq[:, ca:cb],
                            lhsT=cons_h[:, SQW + 128 - 3 * g : SQW + 256 - 3 * g],
                            rhs=sq[:, half, sl],
                            start=(nsq == 0),
                            stop=(nsq == n_mm_per_path - 1),
                        )
                        nsq += 1

        # ---- stacked-stats tail ----
        inv_c = 1.0 / C
        inv_cm1 = 1.0 / (C - 1)

        # Column splits: pipeline the tail stages across ACT/DVE/PE so the
        # post-stream critical path is ~half-width op latencies.
        csls = (
            [slice(0, CHUNK // 2), slice(CHUNK // 2, CHUNK)]
            if tail_split
            else [slice(0, CHUNK)]
        )

        a = tailp.tile([64, CHUNK], F32)  # sum^2 (DVE can't read PSUM twice)
        for cs in csls:
            nc.scalar.activation(a[:, cs], psum_sum[0:64, cs], AF.Square)
        u = tailp.tile([64, CHUNK], BF16)  # sumsq - sum^2/C (sumsq on 3g rows)
        vr = tailp.tile([128, CHUNK], BF16)
        for cs in csls:
            nc.vector.scalar_tensor_tensor(
                u[:, cs], in0=a[:, cs], scalar=-inv_c, in1=psum_sq[0:64, cs],
                op0=OP.mult, op1=OP.add,
            )
            nc.vector.tensor_scalar(
                vr[0:64, cs], in0=u[:, cs], scalar1=cons_f[0:64, 0:1],
                scalar2=1.0, op0=OP.mult, op1=OP.min,
            )
        for cs in csls:
            nc.scalar.activation(vr[64:128, cs], vr[0:64, cs], AF.Sqrt)
        if dummy_sig:
            nc.scalar.activation(psum2[0:2, 0:2], cons_f[0:2, 0:2], AF.Sigmoid)

        # z_k = [W1 pattern] @ [v; r]  (one matmul per hidden half per split)
        nzk = {0: 0, 1: 0}
        for cs in csls:
            for k, (pz, zc) in enumerate(((psum_z0, Z0COL), (psum_z1, Z1COL))):
                nc.tensor.matmul(
                    pz[:, cs], lhsT=cons_h[:, zc : zc + 128], rhs=vr[:, cs],
                    start=(nzk[k] == 0), stop=(nzk[k] == len(csls) - 1),
                )
                nzk[k] += 1
        # h_k = relu(z_k + b1)
        hk0 = tailp.tile([128, CHUNK], BF16)
        hk1 = tailp.tile([128, CHUNK], BF16)
        if tail_split:
            # both halves on DVE: keeps ACT idle after sqrt so the walrus-
            # placed sigmoid table load runs there, off the critical path.
            for cs in csls:
                nc.vector.tensor_scalar(
                    hk0[:, cs], in0=psum_z0[:, cs], scalar1=cons_f[:, 1:2],
                    scalar2=0.0, op0=OP.add, op1=OP.max,
                )
                nc.vector.tensor_scalar(
                    hk1[:, cs], in0=psum_z1[:, cs], scalar1=cons_f[:, 2:3],
                    scalar2=0.0, op0=OP.add, op1=OP.max,
                )
        else:
            nc.vector.tensor_scalar(
                hk0, in0=psum_z0, scalar1=cons_f[:, 1:2], scalar2=0.0,
                op0=OP.add, op1=OP.max,
            )
            if hk1_dve:
                # hk1 on DVE keeps ACT idle after the sqrt, so the walrus-
                # placed sigmoid table load runs there instead of gating
                # the final sigmoid.
                nc.vector.tensor_scalar(
                    hk1, in0=psum_z1, scalar1=cons_f[:, 2:3], scalar2=0.0,
                    op0=OP.add, op1=OP.max,
                )
            else:
                nc.scalar.activation(hk1, psum_z1, AF.Relu, bias=cons_f[:, 2:3])
        nout = 0
        n_out_mm = 2 * len(csls)
        for cs in csls:
            for hk, bdc in ((hk0, BD0COL), (hk1, BD1COL)):
                nc.tensor.matmul(
                    psum2[:, cs], lhsT=cons_h[:, bdc : bdc + NG], rhs=hk[:, cs],
                    start=(nout == 0), stop=(nout == n_out_mm - 1),
                )
                nout += 1
        out_sb = tailp.tile([NG, CHUNK], F32)
        for cs in csls:
            nc.scalar.activation(
                out_sb[:, cs], psum2[:, cs], AF.Sigmoid, bias=cons_f[:NG, 3:4]
            )
        nc.sync.dma_start(out=out_d[:], in_=out_sb)

    nc.finalize()
    return nc


def make_consts(W1, b1, W2, b2):
    C_ = C
    cr = np.zeros((128, 256), np.float32)
    cr[:, 129:131] = 1.0  # sum one-hot: partitions 3g+1, 3g+2 only
    ch = np.zeros((128, NCONST_H), np.float32)
    ch[:, SQW + 128 : SQW + 131] = 1.0  # sq one-hot: partitions 3g..3g+2
    cf = np.zeros((128, NCONST_F), np.float32)
    for g in range(NG):
        cf[3 * g, 0] = 1.0 / C_
        cf[3 * g + 1, 0] = 1.0 / (C_ - 1)
        cf[3 * g + 2, 0] = 1.0 / (C_ - 1)
        for hh in range(8):
            p = 8 * g + hh
            cf[p, 1] = b1[hh]
            cf[p, 2] = b1[8 + hh]
            # stacked W1 pattern: rows = (v;r) partitions, cols = (g,hh)
            for k, zc in ((0, Z0COL), (1, Z1COL)):
                o = 8 * k + hh
                ch[3 * g + 1, zc + p] = W1[1, o]       # var_c from v
                ch[64 + 3 * g, zc + p] = W1[0, o]      # mag_c from r
                ch[64 + 3 * g + 2, zc + p] = W1[2, o]  # grad_c from r
            # block-diag W2
            ch[p, BD0COL + g] = W2[hh, 0]
            ch[p, BD1COL + g] = W2[8 + hh, 0]
    cf[:, 3] = b2[0]
    import ml_dtypes

    ch = ch.astype(ml_dtypes.bfloat16)
    return cr, ch, cf


_CACHE: dict = {}

PIECES5S = [(0, 512), (512, 1024), (1536, 1024), (2560, 1024), (3584, 512)]
PIECES4I = [(0, 1024), (1024, 1024), (2048, 1536), (3584, 512)]
VARIANTS = {
    "A_p5_nodum": dict(pieces=PIECES5, dummy_sig=False),
    "B_p6_dum": dict(pieces=PIECES6, dummy_sig=True),
    "C_p5_dum": dict(pieces=PIECES5, dummy_sig=True),
    "D_p6_nodum": dict(pieces=PIECES6, dummy_sig=False),
    "E_split": dict(pieces=PIECES5, dummy_sig=False, tail_split=True),
    "F_split_fs": dict(pieces=PIECES5S, dummy_sig=False, tail_split=True),
    "H_hk1dve": dict(pieces=PIECES5, hk1_dve=True),
    "I_hk1dve_p4": dict(pieces=PIECES4I, hk1_dve=True),
    "G_dualring": dict(pieces=PIECES5, hk1_dve=True, dual_ring=True),
    "J_gpsq": dict(pieces=PIECES5, hk1_dve=True, gp_square=True),
    "K_dumsqrt": dict(pieces=PIECES5, hk1_dve=True, dummy_sqrt=True),
    "M_J_dumsqrt": dict(pieces=PIECES5, hk1_dve=True, gp_square=True, dummy_sqrt=True),
    "N_hsplit1": dict(pieces=PIECES5, hk1_dve=True, gp_square=True, hsplit_first=True),
}
DEFAULT_VARIANT = "N_hsplit1"


def _get_nc(variant: str = DEFAULT_VARIANT) -> bass.Bass:
    if variant not in _CACHE:
        _CACHE[variant] = build_nc(**VARIANTS[variant])
    return _CACHE[variant]


def run_sharded(features, W1, b1, W2, b2, variant: str = DEFAULT_VARIANT, **spmd_kwargs):
    """Run the SPMD kernel; returns (BassKernelResults, assembled output)."""
    from concourse.bass_utils import run_bass_kernel_spmd

    feats = np.ascontiguousarray(features, dtype=np.float32).reshape(B, C, H * W)
    cr, ch, cf = make_consts(
        np.asarray(W1, np.float32),
        np.asarray(b1, np.float32),
        np.asarray(W2, np.float32),
        np.asarray(b2, np.float32),
    )
    in_maps = [
        {
            "features": np.ascontiguousarray(
                feats[r * B_PER_CORE : (r + 1) * B_PER_CORE]
            ),
            "consts_r": cr,
            "consts_h": ch,
            "consts_f": cf,
        }
        for r in range(NCORES)
    ]
    nc = _get_nc(variant)
    res = run_bass_kernel_spmd(nc, in_maps, core_ids=list(range(NCORES)), **spmd_kwargs)
    out = np.concatenate(
        [res.results[r]["out"].reshape(B_PER_CORE, H, W) for r in range(NCORES)],
        axis=0,
    )
    return res, out


def kernel(features, W1, b1, W2, b2):
    _, out = run_sharded(features, W1, b1, W2, b2)
    return out


# revision 23
# speedup vs baseline: 1.0472x; 1.0472x over previous
"""Trainium2 Bass kernel for nn_FIS_ImportanceAssessment.

Reference computation, per pixel (B=16, C=256, H=W=64):
    sumsq = sum_c f^2 ; sum = sum_c f
    mag   = clip(sqrt(sumsq/C), 0, 1)
    var   = clip((sumsq - sum^2/C)/(C-1), 0, 1)
    grad  = sqrt(var_clipped)               (== clip(sqrt(var), 0, 1))
    out   = sigmoid(relu([mag,var,grad] @ W1 + b1) @ W2 + b2)

Sharding: data-parallel over batch, 2 batches per core across 8 cores.

Per-core layout: the C-axis reduction runs on the PE with "block one-hot"
stationary operands.  The core's 8192 pixels form 16 chunks of 512; chunk
g's column sums land on PSUM partitions {3g, 3g+1, 3g+2} (a "stacked
stats" layout): partition 3g gets only sumsq (mag precursor), 3g+1 and
3g+2 get both sum and sumsq (var / grad precursors).  The tail is then:

    a = sum^2                 (ACT Square; 0 on the 3g rows)
    u = sumsq - a/C           (DVE STT; == sumsq on the 3g rows)
    v = min(alpha*u, 1)       (DVE TS, alpha = 1/C @3g else 1/(C-1))
        -> v[3g]  = clip(sumsq/C)  = mag^2_c,  v[3g+1,2] = var_c
    r = sqrt(v)               (ACT) -> r[3g] = mag_c, r[3g+2] = grad_c
    z_k = W1-block-diag matmul over the stacked (v; r) tile  (PE!)
    h_k = relu(z_k + b1)      (both halves on DVE tensor_scalar)
    out = sigmoid(blockdiag-W2 @ h)                          (PE + ACT)

so the whole 3->16->1 MLP costs 4 matmuls + 2 relu ops instead of a long
per-partition-scalar DVE chain.

ACT table strategy: both relu halves run on DVE, so after the tail sqrt
the ACT engine is idle and the walrus-inserted sigmoid table load (the
only mid-kernel table switch) executes there, off the critical path,
instead of gating the final sigmoid.

Measured (this session): baseline ~47.9 us -> this kernel ~44.1 us.
~10.3 us of that is fixed framework overhead (counted preamble slice +
exit barrier + ~300 event-semaphore resets in the epilogue — present
even for a 3-instruction kernel, measured 15.2 us floor); the feature
stream runs at ~370-410 GB/s, at the per-core HBM roofline.
"""

from contextlib import ExitStack

import numpy as np

import concourse.bacc as bacc
import concourse.bass as bass
import concourse.tile as tile
from concourse import mybir

F32 = mybir.dt.float32
F32R = mybir.dt.float32r  # TF32-style single-pass PE dtype (fp32 is 4 cyc/row)
BF16 = mybir.dt.bfloat16
AF = mybir.ActivationFunctionType
OP = mybir.AluOpType

# -------- problem geometry (hardcoded per contract) --------
B, C, H, W = 16, 256, 64, 64
NCORES = 8
B_PER_CORE = B // NCORES          # 2
PIX = B_PER_CORE * H * W          # 8192 pixels per core
NG = 16                           # pixel chunks ("groups") per core
CHUNK = PIX // NG                 # 512 pixels per chunk (= 1 PSUM bank)
NHID = 16                         # MLP hidden width

# consts_h (bf16) column layout
SQW = 0            # [0:256)    sq one-hot windows (cols 128..130 ones)
Z0COL = 256        # [256:384)  stacked W1 pattern, hidden half 0 (96 rows)
Z1COL = 384        # [384:512)  stacked W1 pattern, hidden half 1
BD0COL = 512       # [512:528)  block-diag W2, half 0
BD1COL = 528       # [528:544)  block-diag W2, half 1
NCONST_H = 544
# consts_f (fp32) column layout
#   0: alpha (1/C on 3g rows, 1/(C-1) on 3g+1,2) ; 1: b1 half0 ; 2: b1 half1
#   3: b2
NCONST_F = 8


PIECES5 = [(0, 1024), (1024, 1024), (2048, 1024), (3072, 512), (3584, 512)]
PIECES6 = [
    (0, 1024),
    (1024, 1024),
    (2048, 1024),
    (3072, 512),
    (3584, 256),
    (3840, 256),
]


def build_nc(pieces=PIECES5, dummy_sig=False, tail_split=False, hk1_dve=False, dual_ring=False, gp_square=False, dummy_sqrt=False, hsplit_first=False, hsplit_all=False) -> bass.Bass:
    # Bacc (not raw Bass): its finalize() runs generate_event_semaphores,
    # which splits multi-sem waits to satisfy the 1-wait-per-instruction
    # hardware constraint that walrus codegen enforces.
    nc = bacc.Bacc()
    # float32r end-to-end for everything the PE consumes: the BIR verifier
    # requires fp32r-matmul inputs to be *produced* as float32r.
    feat = nc.dram_tensor(
        "features", [B_PER_CORE, C, H * W], F32R, kind="ExternalInput"
    )
    cst_r = nc.dram_tensor("consts_r", [128, 256], F32R, kind="ExternalInput")
    cst_h = nc.dram_tensor("consts_h", [128, NCONST_H], BF16, kind="ExternalInput")
    cst_f = nc.dram_tensor("consts_f", [128, NCONST_F], F32, kind="ExternalInput")
    out_d = nc.dram_tensor("out", [NG, CHUNK], F32, kind="ExternalOutput")

    with tile.TileContext(nc) as tc, ExitStack() as ctx:
        singles = ctx.enter_context(tc.tile_pool(name="singles", bufs=1))
        # bufs=2: both streaming rounds get fresh slots, so no x/sq DMA
        # ever carries a buffer-reuse (WAR) wait on top of its RAW wait.
        xpool = ctx.enter_context(tc.tile_pool(name="xpool", bufs=2))
        sqpool = ctx.enter_context(tc.tile_pool(name="sqpool", bufs=2))
        tailp = ctx.enter_context(tc.tile_pool(name="tailp", bufs=1))
        psump = ctx.enter_context(tc.tile_pool(name="psump", bufs=1, space="PSUM"))

        psum_sum = psump.tile([128, CHUNK], F32)
        psum_sq = psump.tile([128, CHUNK], F32)
        psum_z0 = psump.tile([128, CHUNK], F32)
        psum_z1 = psump.tile([128, CHUNK], F32)
        psum2 = psump.tile([NG, CHUNK], F32)

        xs, sqs = [], []
        for b in range(B_PER_CORE):
            xs.append(xpool.tile([128, 2, H * W], F32R, tag="x", name=f"x_{b}"))
            sqs.append(sqpool.tile([128, 2, H * W], BF16, tag="sq", name=f"sq_{b}"))

        # Consts go via SWDGE (gpsimd) so the HWDGE ring is free for the
        # feature stream — features issue immediately after the preamble
        # while the tiny consts land in parallel.
        cons_r = singles.tile([128, 256], F32R)
        nc.gpsimd.dma_start(out=cons_r, in_=cst_r[:])
        cons_h = singles.tile([128, NCONST_H], BF16)
        nc.gpsimd.dma_start(out=cons_h, in_=cst_h[:])
        cons_f = singles.tile([128, NCONST_F], F32)
        nc.gpsimd.dma_start(out=cons_f, in_=cst_f[:])

        if dummy_sqrt:
            # First ACT op = Sqrt: walrus loads the sqrt table set (which
            # also holds square) during the uncounted preamble, so neither
            # the streaming squares nor the tail sqrt pay a table switch.
            # Writes into psum2 (cleared by the start=True output matmul)
            # so DCE can't drop it.
            nc.scalar.activation(psum2[0:2, 2:4], cons_f[0:2, 0:2], AF.Sqrt)

        # Absorb the consts-DMA waits on the PE here so the first real
        # matmuls only wait on the features/squares. (psum2 is cleared again
        # by the real start=True matmul of the MLP output group later.)
        # (2x2, not 1x1: fp32r matmuls require even free dims.)
        nc.tensor.matmul(
            psum2[0:2, 0:2],
            lhsT=cons_r[:, 0:2],
            rhs=cons_r[:, 0:2],
            start=True,
            stop=True,
        )
        nc.tensor.matmul(
            psum2[0:2, 0:2],
            lhsT=cons_h[:, 0:2],
            rhs=cons_h[:, 0:2],
            start=True,
            stop=True,
        )

        # ---- streaming phase: load, square, PE column-sum reductions ----
        # 1 MiB DMA pieces ([128, 2 C-halves, 1024 px]) so compute starts
        # ~3 us in and stays pipelined with the DMA stream.  Squares cast to
        # bf16 (full PE clock + fast weight load on the squared path) and
        # alternate ACT/DVE so no single engine gates the matmul stream.
        sq_engines = {
            (p, h): ("A" if (p + h) % 2 == 0 else "V")
            for p in range(len(pieces))
            for h in range(2)
        }
        # Per-piece matmul sub-plans: (chunk q, column range within the
        # chunk).  256-px pieces produce half-chunk matmuls so the final
        # reductions after the last DMA sem are short.
        def piece_chunks(p0, plen):
            out = []
            for q in range(H * W // CHUNK):
                a0 = max(CHUNK * q, p0)
                b0 = min(CHUNK * (q + 1), p0 + plen)
                if a0 < b0:
                    out.append((q, a0 - CHUNK * q, b0 - CHUNK * q))
            return out

        # batches x C-halves x sub-chunks; same count for the sum and sq paths
        n_mm_per_path = B_PER_CORE * 2 * sum(
            len(piece_chunks(p0, pl)) for p0, pl in pieces
        )
        nsum = 0
        nsq = 0
        npiece = 0
        for b in range(B_PER_CORE):
            x, sq = xs[b], sqs[b]
            feat_b = feat[b].rearrange("(h c) p -> c h p", h=2)
            for p, (p0, plen) in enumerate(pieces):
                psl = slice(p0, p0 + plen)
                # dual_ring: alternate the two physical HWDGE rings (SP and
                # ACT sequencers) so more packets are in flight per SDMA
                # engine.
                deng = nc.scalar if (dual_ring and npiece % 2 == 1) else nc.sync
                if hsplit_all or (hsplit_first and npiece == 0):
                    # Two per-half DMAs for the very first piece: each has
                    # one 4 KiB run per partition instead of two, so HWDGE
                    # issues it in ~half the time and the ring starts
                    # draining earlier — the whole FIFO stream shifts left.
                    deng.dma_start(out=x[:, 0, psl], in_=feat_b[:, 0, psl])
                    deng.dma_start(out=x[:, 1, psl], in_=feat_b[:, 1, psl])
                else:
                    deng.dma_start(out=x[:, :, psl], in_=feat_b[:, :, psl])
                npiece += 1
                last_piece_sq = (
                    gp_square and b == B_PER_CORE - 1 and p == len(pieces) - 1
                )
                for half in range(2):
                    xin = x[:, half, psl].bitcast(F32)
                    sqo = sq[:, half, psl]
                    if last_piece_sq and half == 1:
                        # 3-way split: DVE + GPSIMD halves run in parallel
                        # with ACT's h0 square, shortening the post-stream
                        # reduction lag.
                        hl = plen // 2
                        nc.vector.tensor_mul(
                            sq[:, 1, p0 : p0 + hl],
                            x[:, 1, p0 : p0 + hl].bitcast(F32),
                            x[:, 1, p0 : p0 + hl].bitcast(F32),
                        )
                        nc.gpsimd.tensor_mul(
                            sq[:, 1, p0 + hl : p0 + plen],
                            x[:, 1, p0 + hl : p0 + plen].bitcast(F32),
                            x[:, 1, p0 + hl : p0 + plen].bitcast(F32),
                        )
                    elif sq_engines[(p, half)] == "A":
                        nc.scalar.activation(sqo, xin, AF.Square)
                    else:
                        nc.vector.tensor_mul(sqo, xin, xin)
                # For the final pieces, issue sum-MMs for BOTH halves before
                # the sq-MMs: sum-MMs only need the DMA, so psum_sum (and the
                # tail's a = sum^2) completes earlier while the final squares
                # still run.  Elsewhere keep sum/sq interleaved (global
                # sum-first ordering stalls the PE FIFO).
                last_piece = b == B_PER_CORE - 1 and p >= len(pieces) - 2
                mm_plan = []
                for half in range(2):
                    for qab in piece_chunks(p0, plen):
                        if last_piece:
                            mm_plan.append(("sum", half, qab))
                        else:
                            mm_plan.append(("sum", half, qab))
                            mm_plan.append(("sq", half, qab))
                if last_piece:
                    for half in range(2):
                        for qab in piece_chunks(p0, plen):
                            mm_plan.append(("sq", half, qab))
                for kind, half, (q, ca, cb) in mm_plan:
                    g = b * (H * W // CHUNK) + q
                    sl = slice(q * CHUNK + ca, q * CHUNK + cb)
                    if kind == "sum":
                        nc.tensor.matmul(
                            psum_sum[:, ca:cb],
                            lhsT=cons_r[:, 128 - 3 * g : 256 - 3 * g],
                            rhs=x[:, half, sl],
                            start=(nsum == 0),
                            stop=(nsum == n_mm_per_path - 1),
                        )
                        nsum += 1
                    else:
                        nc.tensor.matmul(
                            psum_sq[:, ca:cb],
                            lhsT=cons_h[:, SQW + 128 - 3 * g : SQW + 256 - 3 * g],
                            rhs=sq[:, half, sl],
                            start=(nsq == 0),
                            stop=(nsq == n_mm_per_path - 1),
                        )
                        nsq += 1

        # ---- stacked-stats tail ----
        inv_c = 1.0 / C
        inv_cm1 = 1.0 / (C - 1)

        # Column splits: pipeline the tail stages across ACT/DVE/PE so the
        # post-stream critical path is ~half-width op latencies.
        csls = (
            [slice(0, CHUNK // 2), slice(CHUNK // 2, CHUNK)]
            if tail_split
            else [slice(0, CHUNK)]
        )

        a = tailp.tile([64, CHUNK], F32)  # sum^2 (DVE can't read PSUM twice)
        for cs in csls:
            nc.scalar.activation(a[:, cs], psum_sum[0:64, cs], AF.Square)
        u = tailp.tile([64, CHUNK], BF16)  # sumsq - sum^2/C (sumsq on 3g rows)
        vr = tailp.tile([128, CHUNK], BF16)
        for cs in csls:
            nc.vector.scalar_tensor_tensor(
                u[:, cs], in0=a[:, cs], scalar=-inv_c, in1=psum_sq[0:64, cs],
                op0=OP.mult, op1=OP.add,
            )
            nc.vector.tensor_scalar(
                vr[0:64, cs], in0=u[:, cs], scalar1=cons_f[0:64, 0:1],
                scalar2=1.0, op0=OP.mult, op1=OP.min,
            )
        for cs in csls:
            nc.scalar.activation(vr[64:128, cs], vr[0:64, cs], AF.Sqrt)
        if dummy_sig:
            nc.scalar.activation(psum2[0:2, 0:2], cons_f[0:2, 0:2], AF.Sigmoid)

        # z_k = [W1 pattern] @ [v; r]  (one matmul per hidden half per split)
        nzk = {0: 0, 1: 0}
        for cs in csls:
            for k, (pz, zc) in enumerate(((psum_z0, Z0COL), (psum_z1, Z1COL))):
                nc.tensor.matmul(
                    pz[:, cs], lhsT=cons_h[:, zc : zc + 128], rhs=vr[:, cs],
                    start=(nzk[k] == 0), stop=(nzk[k] == len(csls) - 1),
                )
                nzk[k] += 1
        # h_k = relu(z_k + b1)
        hk0 = tailp.tile([128, CHUNK], BF16)
        hk1 = tailp.tile([128, CHUNK], BF16)
        if tail_split:
            # both halves on DVE: keeps ACT idle after sqrt so the walrus-
            # placed sigmoid table load runs there, off the critical path.
            for cs in csls:
                nc.vector.tensor_scalar(
                    hk0[:, cs], in0=psum_z0[:, cs], scalar1=cons_f[:, 1:2],
                    scalar2=0.0, op0=OP.add, op1=OP.max,
                )
                nc.vector.tensor_scalar(
                    hk1[:, cs], in0=psum_z1[:, cs], scalar1=cons_f[:, 2:3],
                    scalar2=0.0, op0=OP.add, op1=OP.max,
                )
        else:
            nc.vector.tensor_scalar(
                hk0, in0=psum_z0, scalar1=cons_f[:, 1:2], scalar2=0.0,
                op0=OP.add, op1=OP.max,
            )
            if hk1_dve:
                # hk1 on DVE keeps ACT idle after the sqrt, so the walrus-
                # placed sigmoid table load runs there instead of gating
                # the final sigmoid.
                nc.vector.tensor_scalar(
                    hk1, in0=psum_z1, scalar1=cons_f[:, 2:3], scalar2=0.0,
                    op0=OP.add, op1=OP.max,
                )
            else:
                nc.scalar.activation(hk1, psum_z1, AF.Relu, bias=cons_f[:, 2:3])
        nout = 0
        n_out_mm = 2 * len(csls)
        for cs in csls:
            for hk, bdc in ((hk0, BD0COL), (hk1, BD1COL)):
                nc.tensor.matmul(
                    psum2[:, cs], lhsT=cons_h[:, bdc : bdc + NG], rhs=hk[:, cs],
                    start=(nout == 0), stop=(nout == n_out_mm - 1),
                )
                nout += 1
        out_sb = tailp.tile([NG, CHUNK], F32)
        for cs in csls:
            nc.scalar.activation(
                out_sb[:, cs], psum2[:, cs], AF.Sigmoid, bias=cons_f[:NG, 3:4]
            )
        nc.sync.dma_start(out=out_d[:], in_=out_sb)

    nc.finalize()
    return nc


def make_consts(W1, b1, W2, b2):
    C_ = C
    cr = np.zeros((128, 256), np.float32)
    cr[:, 129:131] = 1.0  # sum one-hot: partitions 3g+1, 3g+2 only
    ch = np.zeros((128, NCONST_H), np.float32)
    ch[:, SQW + 128 : SQW + 131] = 1.0  # sq one-hot: partitions 3g..3g+2
    cf = np.zeros((128, NCONST_F), np.float32)
    for g in range(NG):
        cf[3 * g, 0] = 1.0 / C_
        cf[3 * g + 1, 0] = 1.0 / (C_ - 1)
        cf[3 * g + 2, 0] = 1.0 / (C_ - 1)
        for hh in range(8):
            p = 8 * g + hh
            cf[p, 1] = b1[hh]
            cf[p, 2] = b1[8 + hh]
            # stacked W1 pattern: rows = (v;r) partitions, cols = (g,hh)
            for k, zc in ((0, Z0COL), (1, Z1COL)):
                o = 8 * k + hh
                ch[3 * g + 1, zc + p] = W1[1, o]       # var_c from v
                ch[64 + 3 * g, zc + p] = W1[0, o]      # mag_c from r
                ch[64 + 3 * g + 2, zc + p] = W1[2, o]  # grad_c from r
            # block-diag W2
            ch[p, BD0COL + g] = W2[hh, 0]
            ch[p, BD1COL + g] = W2[8 + hh, 0]
    cf[:, 3] = b2[0]
    import ml_dtypes

    ch = ch.astype(ml_dtypes.bfloat16)
    return cr, ch, cf


_CACHE: dict = {}

PIECES5S = [(0, 512), (512, 1024), (1536, 1024), (2560, 1024), (3584, 512)]
PIECES4I = [(0, 1024), (1024, 1024), (2048, 1536), (3584, 512)]
VARIANTS = {
    "A_p5_nodum": dict(pieces=PIECES5, dummy_sig=False),
    "B_p6_dum": dict(pieces=PIECES6, dummy_sig=True),
    "C_p5_dum": dict(pieces=PIECES5, dummy_sig=True),
    "D_p6_nodum": dict(pieces=PIECES6, dummy_sig=False),
    "E_split": dict(pieces=PIECES5, dummy_sig=False, tail_split=True),
    "F_split_fs": dict(pieces=PIECES5S, dummy_sig=False, tail_split=True),
    "H_hk1dve": dict(pieces=PIECES5, hk1_dve=True),
    "I_hk1dve_p4": dict(pieces=PIECES4I, hk1_dve=True),
    "G_dualring": dict(pieces=PIECES5, hk1_dve=True, dual_ring=True),
    "J_gpsq": dict(pieces=PIECES5, hk1_dve=True, gp_square=True),
    "K_dumsqrt": dict(pieces=PIECES5, hk1_dve=True, dummy_sqrt=True),
    "M_J_dumsqrt": dict(pieces=PIECES5, hk1_dve=True, gp_square=True, dummy_sqrt=True),
    "N_hsplit1": dict(pieces=PIECES5, hk1_dve=True, gp_square=True, hsplit_first=True),
    "O_hsplitall": dict(pieces=PIECES5, hk1_dve=True, gp_square=True, hsplit_all=True),
}
DEFAULT_VARIANT = "O_hsplitall"


def _get_nc(variant: str = DEFAULT_VARIANT) -> bass.Bass:
    if variant not in _CACHE:
        _CACHE[variant] = build_nc(**VARIANTS[variant])
    return _CACHE[variant]


def run_sharded(features, W1, b1, W2, b2, variant: str = DEFAULT_VARIANT, **spmd_kwargs):
    """Run the SPMD kernel; returns (BassKernelResults, assembled output)."""
    from concourse.bass_utils import run_bass_kernel_spmd

    feats = np.ascontiguousarray(features, dtype=np.float32).reshape(B, C, H * W)
    cr, ch, cf = make_consts(
        np.asarray(W1, np.float32),
        np.asarray(b1, np.float32),
        np.asarray(W2, np.float32),
        np.asarray(b2, np.float32),
    )
    in_maps = [
        {
            "features": np.ascontiguousarray(
                feats[r * B_PER_CORE : (r + 1) * B_PER_CORE]
            ),
            "consts_r": cr,
            "consts_h": ch,
            "consts_f": cf,
        }
        for r in range(NCORES)
    ]
    nc = _get_nc(variant)
    res = run_bass_kernel_spmd(nc, in_maps, core_ids=list(range(NCORES)), **spmd_kwargs)
    out = np.concatenate(
        [res.results[r]["out"].reshape(B_PER_CORE, H, W) for r in range(NCORES)],
        axis=0,
    )
    return res, out


def kernel(features, W1, b1, W2, b2):
    _, out = run_sharded(features, W1, b1, W2, b2)
    return out
